# revision 16
# baseline (speedup 1.0000x reference)
"""DeepGraphSAGE Trainium2 kernel (8 NeuronCores, data-parallel over graphs).

v2: fp8 inputs (x, raw-count adjacency), inv-degree in epilogues,
two-graph software pipelining to keep the tensor engine busy, epilogues
spread across Vector/Scalar/GpSimd engines.

Sharding: 512 graphs -> 64 per core; edges never cross graphs. Per graph
the 400x400 adjacency ships as raw edge counts (exact in fp8e4m3) in
PE-tile layout; node features ship transposed fp8; weights replicated.
"""

import sys

sys.path.insert(0, "/opt/trn_rl_repo")

import numpy as np
import ml_dtypes

import concourse.bass as bass
import concourse.bacc as bacc
import concourse.mybir as mybir
from concourse.tile import TileContext
from concourse.bass_utils import run_bass_kernel_spmd

BF16 = ml_dtypes.bfloat16
F8 = ml_dtypes.float8_e4m3fn
F32 = mybir.dt.float32
B16 = mybir.dt.bfloat16
E4 = mybir.dt.float8e4

NCORES = 8
B = 512          # graphs
NPG = 400        # nodes per graph
NP = 512         # padded nodes per graph
EPG = 6400       # edges per graph
F_IN = 200       # input feature dim
H = 64           # hidden
NCH = 4          # node chunks of 128
G = B // NCORES  # graphs per core

AX = mybir.AxisListType.X
OP = mybir.AluOpType
AF = mybir.ActivationFunctionType


def ts(i, n):
    return slice(i * n, (i + 1) * n)


# ----------------------------------------------------------------------------
# Device kernel
# ----------------------------------------------------------------------------

def build_kernel(g_count=G, n_bisect=24, dbg=False):
    nc = bacc.Bacc("TRN2", debug=False)

    xa_d = nc.declare_dram_parameter("xa", [g_count, 128, NP], E4, isOutput=False)
    xb_d = nc.declare_dram_parameter("xb", [g_count, 72, NP], E4, isOutput=False)
    adj_d = nc.declare_dram_parameter("adj", [g_count, 128, 2048], E4, isOutput=False)
    invd_d = nc.declare_dram_parameter("invd", [128, g_count * 4], F32, isOutput=False)
    cb_d = nc.declare_dram_parameter("cb16", [128, 520], B16, isOutput=False)
    cf_d = nc.declare_dram_parameter("cf32", [128, 128], F32, isOutput=False)
    out_d = nc.declare_dram_parameter("out", [2, g_count], F32, isOutput=True)
    if dbg:
        dbg_u1 = nc.declare_dram_parameter("dbg_u1", [128, 256], F32, isOutput=True)
        dbg_v1 = nc.declare_dram_parameter("dbg_v1", [128, 256], F32, isOutput=True)
        dbg_h1 = nc.declare_dram_parameter("dbg_h1", [128, 512], F32, isOutput=True)
        dbg_h2 = nc.declare_dram_parameter("dbg_h2", [128, 512], F32, isOutput=True)
        dbg_h3 = nc.declare_dram_parameter("dbg_h3", [128, 512], F32, isOutput=True)
        dbg_sS = nc.declare_dram_parameter("dbg_sS", [64, 512], F32, isOutput=True)
        dbg_lo = nc.declare_dram_parameter("dbg_lo", [64, 1], F32, isOutput=True)
        dbg_w = nc.declare_dram_parameter("dbg_w", [64, 512], F32, isOutput=True)
        dbg_pf = nc.declare_dram_parameter("dbg_pf", [65, 64], F32, isOutput=True)

    with TileContext(nc) as tc:
        with (
            tc.tile_pool(name="const", bufs=1) as cpool,
            tc.tile_pool(name="xp", bufs=4) as xpool,
            tc.tile_pool(name="ap", bufs=4) as apool,
            tc.tile_pool(name="up", bufs=4) as upool,
            tc.tile_pool(name="hp", bufs=5) as hpool,
            tc.tile_pool(name="zp", bufs=4) as zpool,
            tc.tile_pool(name="kp", bufs=g_count) as kpool,
            tc.tile_pool(name="puvp", bufs=2, space="PSUM") as puvp,
            tc.tile_pool(name="paggp", bufs=2, space="PSUM") as paggp,
            tc.tile_pool(name="pscr", bufs=3, space="PSUM") as pscr,
            tc.tile_pool(name="ppers", bufs=1, space="PSUM") as ppers,
        ):
            # ---- constants ----
            cb = cpool.tile([128, 520], B16, tag="cb")
            nc.sync.dma_start(out=cb[:], in_=cb_d[:])
            cf = cpool.tile([128, 128], F32, tag="cf")
            nc.sync.dma_start(out=cf[:], in_=cf_d[:])
            invd_sb = cpool.tile([128, g_count * 4], F32, tag="invd")
            nc.sync.dma_start(out=invd_sb[:], in_=invd_d[:])

            ident = cb[:, 0:128]          # I128 bf16
            w1a = cb[:, 128:256]          # [128,128] W1cat rows 0:128
            w1b = cb[0:72, 256:384]       # [72,128] W1cat rows 128:200
            w2 = cb[:, 384:448]           # [128,64] [W2r;W2l]
            w3 = cb[:, 448:512]           # [128,64]
            wp = cb[:, 512:513]           # [128,1] [Wpo;Wpr]
            wlin = cb[0:65, 516:518]      # [65,2] [Wlin;blin]
            identf = cf[:, 0:128]         # fp32 I128

            pooled_ps = ppers.tile([64, 64], F32, tag="pooled")

            # node-major scores: s_all[p, 4g+c]
            s_all = cpool.tile([128, 256], F32, tag="sall")
            nc.vector.memset(s_all[:], 0.0)

            # ---------------- per-graph stage emitters ----------------
            state = {}

            def st(g):
                return state.setdefault(g, {})

            def stage_dma(g):
                s = st(g)
                s["xa"] = xpool.tile([128, NP], E4, tag="xa", name="xat")
                nc.sync.dma_start(out=s["xa"][:], in_=xa_d[g])
                s["xb"] = xpool.tile([72, NP], E4, tag="xb", name="xbt")
                nc.sync.dma_start(out=s["xb"][:], in_=xb_d[g])
                s["a"] = apool.tile([128, 2048], E4, tag="a", name="at")
                nc.sync.dma_start(out=s["a"][:], in_=adj_d[g])

            def a_tile(g, kc, mc):
                return st(g)["a"][:, kc * 512 + mc * 128: kc * 512 + (mc + 1) * 128]

            def stage_l1(g):
                # u|v = x @ [W1l|W1r]; u -> bf16 sbuf (DVE), v -> bf16 sbuf (ACT)
                s = st(g)
                puv = puvp.tile([128, 4, 128], F32, tag="puv")
                for c in range(NCH):
                    nc.tensor.matmul(puv[:, c, :], s["xa"][:, ts(c, 128)], w1a,
                                     start=True, stop=False)
                    nc.tensor.matmul(puv[:, c, :], s["xb"][:, ts(c, 128)], w1b,
                                     start=False, stop=True)
                u1 = upool.tile([128, 4, H], B16, tag="u1")
                nc.vector.tensor_copy(u1[:], puv[:, :, 0:H])
                v1 = upool.tile([128, 4, H], B16, tag="v1")
                nc.scalar.activation(v1[:], puv[:, :, H:128], AF.Copy)
                s["u1"], s["v1"] = u1, v1

            def stage_agg(g, src_tile, out_key):
                # sum-aggregation: pA[dst] = sum_src A_raw[src,dst] * src[src]
                s = st(g)
                pA = paggp.tile([128, 4, H], F32, tag="agg")
                for mc in range(NCH):
                    for kc in range(NCH):
                        nc.tensor.matmul(pA[:, mc, :], a_tile(g, kc, mc),
                                         src_tile[:, kc, 0:H],
                                         start=(kc == 0), stop=(kc == NCH - 1))
                s[out_key] = pA

            def invd_ap(g):
                return invd_sb[:, g * 4: g * 4 + 4]

            def ivbc(g):
                return invd_ap(g).unsqueeze(2).broadcast_to([128, 4, H])

            def epi_l1(g):
                # h1 = relu(sum1 * invd + v1): TT (DVE) then add+relu (GpSimd)
                s = st(g)
                tmp = upool.tile([128, 4, H], B16, tag="t1")
                nc.vector.tensor_tensor(tmp[:], s["p1"][:], ivbc(g), OP.mult)
                tmp2 = upool.tile([128, 4, H], B16, tag="t2")
                nc.gpsimd.tensor_tensor(tmp2[:], tmp[:], s["v1"][:], OP.add)
                hcat = hpool.tile([128, 4, 128], B16, tag="hcat")
                nc.gpsimd.tensor_scalar(hcat[:, :, 0:H], tmp2[:], 0.0, None, OP.max)
                s["hc1"] = hcat

            def epi_mean(g, pkey, hckey, last=False):
                # mean-agg copy into cat bottom half (ACT, per-partition scale)
                s = st(g)
                iv = invd_ap(g)
                hc = s[hckey]
                if last:
                    # score layer needs the RAW sum-aggregation (no 1/deg)
                    nc.vector.tensor_copy(hc[:, :, H:128], s[pkey][:])
                    return
                nc.vector.tensor_tensor(hc[:, :, H:128], s[pkey][:], ivbc(g),
                                        OP.mult)

            def stage_tz(g, hckey, wcat, out_hckey, layer):
                # transpose cat -> zt (GpSimd copy), z matmul, epilogue STT (DVE)
                s = st(g)
                hc = s[hckey]
                pT = pscr.tile([128, 512], B16, tag="ps", name="pT")
                for c in range(NCH):
                    nc.tensor.transpose(pT[:, ts(c, 128)], hc[:, c, :], ident)
                zt = zpool.tile([128, 512], B16, tag="zt")
                if zt_init[0] < 4:
                    zt_init[0] += 1
                    nc.gpsimd.memset(zt[:, 400:512], 0.0)
                nc.scalar.activation(zt[:, 0:400], pT[:, 0:400], AF.Copy)
                pZ = pscr.tile([128, 4, H], F32, tag="ps", name="pZ")
                for c in range(NCH):
                    nc.tensor.matmul(pZ[:, c, :], zt[:, ts(c, 128)], wcat,
                                     start=True, stop=True)
                if layer == 3:
                    hn = kpool.tile([128, 4, 128], B16, tag="h3k")
                    h3list.append(hn)
                else:
                    hn = hpool.tile([128, 4, 128], B16, tag="hcat")
                nc.vector.scalar_tensor_tensor(
                    hn[:, :, 0:H], pZ[:], 0.0, hc[:, :, 0:H], OP.max, OP.add)
                s[out_hckey] = hn

            def stage_score_z(g):
                # scores: s = catS^T @ [Wpo;Wpr] (catS = [h3 | raw sum-agg])
                s = st(g)
                hc = s["hc3"]
                pT = pscr.tile([128, 512], B16, tag="ps", name="pT")
                for c in range(NCH):
                    nc.tensor.transpose(pT[:, ts(c, 128)], hc[:, c, :], ident)
                zt = zpool.tile([128, 512], B16, tag="zt")
                if zt_init[0] < 4:
                    zt_init[0] += 1
                    nc.gpsimd.memset(zt[:, 400:512], 0.0)
                nc.scalar.activation(zt[:, 0:400], pT[:, 0:400], AF.Copy)
                s_ps = pscr.tile([128, 4], F32, tag="ps", name="s_ps")
                for c in range(NCH):
                    nc.tensor.matmul(s_ps[:, c:c + 1], zt[:, ts(c, 128)], wp,
                                     start=True, stop=True)
                nc.vector.tensor_scalar(s_all[:, g:256:64], s_ps[:], 0.0, None,
                                        OP.add)

            # ---------------- interleaved graph-group loop (3-deep) ----------------
            h3list = []
            zt_init = [0]
            groups = [list(range(i, i + 3)) for i in range(0, 60, 3)] + [[60, 61, 62, 63]]
            assert sum(len(gr) for gr in groups) == g_count
            for gi, gr in enumerate(groups):
                if gi == 0:
                    for g in gr:
                        stage_dma(g)
                if gi + 1 < len(groups):
                    for g in groups[gi + 1]:
                        stage_dma(g)
                for g in gr:
                    stage_l1(g)
                for g in gr:
                    stage_agg(g, st(g)["u1"], "p1")
                    epi_l1(g)
                for g in gr:
                    stage_agg(g, st(g)["hc1"], "p2")
                    epi_mean(g, "p2", "hc1")
                for g in gr:
                    stage_tz(g, "hc1", w2, "hc2", 2)
                for g in gr:
                    stage_agg(g, st(g)["hc2"], "p3")
                    epi_mean(g, "p3", "hc2")
                for g in gr:
                    stage_tz(g, "hc2", w3, "hc3", 3)
                for g in gr:
                    stage_agg(g, st(g)["hc3"], "pS")
                    epi_mean(g, "pS", "hc3", last=True)
                for g in gr:
                    stage_score_z(g)
                if dbg and 0 in gr:
                    a = 0
                    nc.gpsimd.dma_start(out=dbg_u1[:], in_=st(a)["u1"][:])
                    nc.gpsimd.dma_start(out=dbg_v1[:], in_=st(a)["v1"][:])
                    nc.gpsimd.dma_start(out=dbg_h1[:], in_=st(a)["hc1"][:])
                    nc.gpsimd.dma_start(out=dbg_h2[:], in_=st(a)["hc2"][:])
                    nc.gpsimd.dma_start(out=dbg_h3[:], in_=st(a)["hc3"][:])
                for g in gr:
                    st(g).clear()

            # ------------------------- top-k threshold -------------------------
            S = cpool.tile([64, 512], F32, tag="S")
            for c in range(4):
                pTs = pscr.tile([64, 128], F32, tag="ps", name="pTs")
                nc.tensor.transpose(pTs[:], s_all[:, ts(c, 64)], identf)
                nc.vector.tensor_copy(S[:, ts(c, 128)], pTs[:])
            nc.vector.memset(S[:, 400:512], -1e30)
            ones400 = cpool.tile([64, 400], F32, tag="ones400")
            nc.vector.memset(ones400[:], 1.0)
            cmp_s = cpool.tile([64, 400], F32, tag="cmps")
            lo = cpool.tile([64, 1], F32, tag="lo")
            hi = cpool.tile([64, 1], F32, tag="hi")
            mid = cpool.tile([64, 1], F32, tag="mid")
            cnt = cpool.tile([64, 1], F32, tag="cnt")
            msk = cpool.tile([64, 1], mybir.dt.uint8, tag="msk")
            msk2 = cpool.tile([64, 1], mybir.dt.uint8, tag="msk2")
            nc.vector.tensor_reduce(lo[:], S[:, 0:400], AX, OP.min)
            nc.vector.tensor_scalar(lo[:], lo[:], -1.0, None, OP.add)
            nc.vector.tensor_reduce(hi[:], S[:, 0:400], AX, OP.max)
            nc.vector.tensor_scalar(hi[:], hi[:], 1.0, None, OP.add)
            for _ in range(n_bisect):
                nc.vector.tensor_tensor(mid[:], lo[:], hi[:], OP.add)
                nc.vector.tensor_scalar(mid[:], mid[:], 0.5, None, OP.mult)
                nc.vector.scalar_tensor_tensor(
                    cmp_s[:], S[:, 0:400], mid[:], ones400[:], OP.is_ge, OP.mult,
                    accum_out=cnt[:])
                nc.vector.tensor_scalar(msk[:], cnt[:], 200.0, None, OP.is_ge)
                nc.vector.tensor_scalar(msk2[:], cnt[:], 200.0, None, OP.is_lt)
                nc.vector.select(lo[:], msk[:], mid[:], lo[:])
                nc.vector.select(hi[:], msk2[:], mid[:], hi[:])

            if dbg:
                nc.gpsimd.dma_start(out=dbg_sS[:], in_=S[:])
            # w = tanh(s) * (s >= thresh)   (graph-major)
            tnh = cpool.tile([64, 512], F32, tag="tnh")
            nc.scalar.activation(tnh[:], S[:], AF.Tanh)
            wgm = cpool.tile([64, 512], B16, tag="wgm")
            nc.vector.scalar_tensor_tensor(
                wgm[:], S[:], lo[:], tnh[:], OP.is_ge, OP.mult)
            if dbg:
                nc.gpsimd.dma_start(out=dbg_lo[:], in_=lo[:])
                nc.gpsimd.dma_start(out=dbg_w[:], in_=wgm[:])
            pw = pscr.tile([128, 4, H], B16, tag="ps", name="pw")
            for c in range(NCH):
                nc.tensor.transpose(pw[:, c, :], wgm[:, ts(c, 128)],
                                    ident[0:64, 0:64])
            w_all = cpool.tile([128, 4, H], B16, tag="wall")
            nc.scalar.activation(w_all[:], pw[:], AF.Copy)

            # ------------------------- pooling + classifier --------------------
            if g_count < 64:
                nc.vector.memset(pooled_ps[:], 0.0)
            for g in range(g_count):
                h3k = h3list[g]
                for c in range(NCH):
                    nc.tensor.matmul(pooled_ps[:, g:g + 1], h3k[:, c, 0:H],
                                     w_all[:, c, g:g + 1],
                                     start=(c == 0), stop=(c == NCH - 1))
            pool_fm = cpool.tile([65, 64], B16, tag="poolfm")
            nc.vector.memset(pool_fm[64:65, :], 1.0)
            nc.scalar.activation(pool_fm[0:64, :], pooled_ps[:], AF.Copy,
                                 scale=1.0 / 200.0)
            if dbg:
                nc.gpsimd.dma_start(out=dbg_pf[:], in_=pool_fm[:])
            plw = pscr.tile([1, 128], F32, tag="ps", name="plw")
            for cls in range(2):
                nc.tensor.matmul(plw[0:1, ts(cls, 64)], wlin[:, cls:cls + 1],
                                 pool_fm[:], start=True, stop=True)
            lgw = cpool.tile([1, 128], F32, tag="lgw")
            nc.vector.tensor_copy(lgw[:], plw[:])
            m01 = cpool.tile([1, 64], F32, tag="m01")
            d0 = cpool.tile([1, 64], F32, tag="d0")
            d1 = cpool.tile([1, 64], F32, tag="d1")
            e0 = cpool.tile([1, 64], F32, tag="e0")
            e1 = cpool.tile([1, 64], F32, tag="e1")
            lse = cpool.tile([1, 64], F32, tag="lse")
            out_sb = cpool.tile([1, 128], F32, tag="outsb")
            nc.vector.tensor_tensor(m01[:], lgw[:, 0:64], lgw[:, 64:128], OP.max)
            nc.vector.tensor_tensor(d0[:], lgw[:, 0:64], m01[:], OP.subtract)
            nc.vector.tensor_tensor(d1[:], lgw[:, 64:128], m01[:], OP.subtract)
            nc.scalar.activation(e0[:], d0[:], AF.Exp)
            nc.scalar.activation(e1[:], d1[:], AF.Exp)
            nc.vector.tensor_tensor(lse[:], e0[:], e1[:], OP.add)
            nc.scalar.activation(lse[:], lse[:], AF.Ln)
            nc.vector.tensor_tensor(out_sb[:, 0:64], d0[:], lse[:], OP.subtract)
            nc.vector.tensor_tensor(out_sb[:, 64:128], d1[:], lse[:], OP.subtract)
            ov = out_sb[:].rearrange("p (a b) -> p a b", a=2)[:, :, 0:g_count]
            nc.sync.dma_start(out=out_d[:], in_=ov)

    nc.compile()
    return nc


# ----------------------------------------------------------------------------
# Host-side shard/layout prep
# ----------------------------------------------------------------------------

def _prep(x, edge_index, W1l, W1r, W2l, W2r, W3l, W3r, Wpr, Wpo, Wlin, blin,
          n_graphs=B):
    src = np.asarray(edge_index[0]) % NPG
    dst = np.asarray(edge_index[1]) % NPG
    key = (src.astype(np.int64) * NPG + dst).reshape(n_graphs, EPG)

    A = np.zeros((n_graphs, NPG * NPG), np.float32)
    for g in range(n_graphs):
        A[g] = np.bincount(key[g], minlength=NPG * NPG)
    A = A.reshape(n_graphs, NPG, NPG)          # A[g, s, d] = edge count s->d
    deg = A.sum(axis=1)                        # in-degree per dst [g, 400]
    inv = (1.0 / np.maximum(deg, 1.0)).astype(np.float32)
    Ap = np.zeros((n_graphs, NP, NP), np.float32)
    Ap[:, :NPG, :NPG] = A
    adj = np.ascontiguousarray(
        Ap.reshape(n_graphs, 4, 128, 4, 128).transpose(0, 2, 1, 3, 4)
        .reshape(n_graphs, 128, 2048)).astype(F8)

    invp = np.zeros((n_graphs, NP), np.float32)
    invp[:, :NPG] = inv
    inv_nm = np.ascontiguousarray(
        invp.reshape(n_graphs, 4, 128).transpose(2, 0, 1)
        .reshape(128, n_graphs * 4))

    x = np.asarray(x, np.float32)
    xT = np.zeros((n_graphs, F_IN, NP), np.float32)
    xT[:, :, :NPG] = x.reshape(n_graphs, NPG, F_IN).transpose(0, 2, 1)
    xa = np.ascontiguousarray(xT[:, 0:128, :]).astype(F8)
    xb = np.ascontiguousarray(xT[:, 128:200, :]).astype(F8)

    def n_(a):
        return np.asarray(a, np.float32)

    cb16 = np.zeros((128, 520), np.float32)
    cb16[:, 0:128] = np.eye(128)
    w1cat = np.concatenate([n_(W1l), n_(W1r)], axis=1)       # [200, 128]
    cb16[:, 128:256] = w1cat[0:128]
    cb16[0:72, 256:384] = w1cat[128:200]
    cb16[:, 384:448] = np.concatenate([n_(W2r), n_(W2l)], axis=0)
    cb16[:, 448:512] = np.concatenate([n_(W3r), n_(W3l)], axis=0)
    cb16[:, 512:513] = np.concatenate([n_(Wpo), n_(Wpr)], axis=0)
    cb16[0:64, 516:518] = n_(Wlin)
    cb16[64, 516:518] = n_(blin)
    cb16 = cb16.astype(BF16)

    cf32 = np.eye(128, dtype=np.float32)

    return xa, xb, adj, inv_nm, cb16, cf32


def kernel(**inputs):
    xa, xb, adj, inv_nm, cb16, cf32 = _prep(
        inputs["x"], inputs["edge_index"], inputs["W1l"], inputs["W1r"],
        inputs["W2l"], inputs["W2r"], inputs["W3l"], inputs["W3r"],
        inputs["Wpr"], inputs["Wpo"], inputs["Wlin"], inputs["blin"])

    nc = build_kernel(G)

    in_maps = []
    for c in range(NCORES):
        gs = slice(c * G, (c + 1) * G)
        in_maps.append({
            "xa": np.ascontiguousarray(xa[gs]),
            "xb": np.ascontiguousarray(xb[gs]),
            "adj": np.ascontiguousarray(adj[gs]),
            "invd": np.ascontiguousarray(inv_nm[:, c * G * 4:(c + 1) * G * 4]),
            "cb16": cb16,
            "cf32": cf32,
        })
    res = run_bass_kernel_spmd(nc, in_maps, list(range(NCORES)))
    outs = [res.results[i]["out"] for i in range(NCORES)]    # each [2, G]
    logits = np.concatenate(outs, axis=1).T                  # [512, 2]
    return np.ascontiguousarray(logits.astype(np.float32))


# revision 18
# speedup vs baseline: 1.3895x; 1.3895x over previous
"""DeepGraphSAGE Trainium2 kernel (8 NeuronCores, data-parallel over graphs).

v2: fp8 inputs (x, raw-count adjacency), inv-degree in epilogues,
two-graph software pipelining to keep the tensor engine busy, epilogues
spread across Vector/Scalar/GpSimd engines.

Sharding: 512 graphs -> 64 per core; edges never cross graphs. Per graph
the 400x400 adjacency ships as raw edge counts (exact in fp8e4m3) in
PE-tile layout; node features ship transposed fp8; weights replicated.
"""

import sys

sys.path.insert(0, "/opt/trn_rl_repo")

import numpy as np
import ml_dtypes

import concourse.bass as bass
import concourse.bacc as bacc
import concourse.mybir as mybir
from concourse.tile import TileContext
from concourse.bass_utils import run_bass_kernel_spmd

BF16 = ml_dtypes.bfloat16
F8 = ml_dtypes.float8_e4m3fn
F32 = mybir.dt.float32
B16 = mybir.dt.bfloat16
E4 = mybir.dt.float8e4

NCORES = 8
B = 512          # graphs
NPG = 400        # nodes per graph
NP = 512         # padded nodes per graph
EPG = 6400       # edges per graph
F_IN = 200       # input feature dim
H = 64           # hidden
NCH = 4          # node chunks of 128
G = B // NCORES  # graphs per core

AX = mybir.AxisListType.X
OP = mybir.AluOpType
AF = mybir.ActivationFunctionType


def ts(i, n):
    return slice(i * n, (i + 1) * n)


# ----------------------------------------------------------------------------
# Device kernel
# ----------------------------------------------------------------------------

def build_kernel(g_count=G, n_bisect=24, dbg=False):
    nc = bacc.Bacc("TRN2", debug=False)

    xa_d = nc.declare_dram_parameter("xa", [g_count, 128, 400], E4, isOutput=False)
    xb_d = nc.declare_dram_parameter("xb", [g_count, 72, 400], E4, isOutput=False)
    adj_d = nc.declare_dram_parameter("adj", [g_count, 128, 1600], E4, isOutput=False)
    invd_d = nc.declare_dram_parameter("invd", [128, g_count * 4], F32, isOutput=False)
    cb_d = nc.declare_dram_parameter("cb16", [128, 520], B16, isOutput=False)
    cf_d = nc.declare_dram_parameter("cf32", [128, 128], F32, isOutput=False)
    out_d = nc.declare_dram_parameter("out", [2, g_count], F32, isOutput=True)
    if dbg:
        dbg_u1 = nc.declare_dram_parameter("dbg_u1", [128, 256], F32, isOutput=True)
        dbg_v1 = nc.declare_dram_parameter("dbg_v1", [128, 256], F32, isOutput=True)
        dbg_h1 = nc.declare_dram_parameter("dbg_h1", [128, 512], F32, isOutput=True)
        dbg_h2 = nc.declare_dram_parameter("dbg_h2", [128, 512], F32, isOutput=True)
        dbg_h3 = nc.declare_dram_parameter("dbg_h3", [128, 512], F32, isOutput=True)
        dbg_sS = nc.declare_dram_parameter("dbg_sS", [64, 512], F32, isOutput=True)
        dbg_lo = nc.declare_dram_parameter("dbg_lo", [64, 1], F32, isOutput=True)
        dbg_w = nc.declare_dram_parameter("dbg_w", [64, 512], F32, isOutput=True)
        dbg_pf = nc.declare_dram_parameter("dbg_pf", [65, 64], F32, isOutput=True)

    with TileContext(nc) as tc:
        with (
            tc.tile_pool(name="const", bufs=1) as cpool,
            tc.tile_pool(name="xp", bufs=4) as xpool,
            tc.tile_pool(name="ap", bufs=4) as apool,
            tc.tile_pool(name="up", bufs=4) as upool,
            tc.tile_pool(name="hp", bufs=5) as hpool,
            tc.tile_pool(name="zp", bufs=4) as zpool,
            tc.tile_pool(name="kp", bufs=g_count) as kpool,
            tc.tile_pool(name="puvp", bufs=2, space="PSUM") as puvp,
            tc.tile_pool(name="paggp", bufs=2, space="PSUM") as paggp,
            tc.tile_pool(name="pscr", bufs=3, space="PSUM") as pscr,
            tc.tile_pool(name="ppers", bufs=1, space="PSUM") as ppers,
        ):
            # ---- constants ----
            cb = cpool.tile([128, 520], B16, tag="cb")
            nc.sync.dma_start(out=cb[:], in_=cb_d[:])
            cf = cpool.tile([128, 128], F32, tag="cf")
            nc.sync.dma_start(out=cf[:], in_=cf_d[:])
            invd_sb = cpool.tile([128, g_count * 4], F32, tag="invd")
            nc.sync.dma_start(out=invd_sb[:], in_=invd_d[:])

            ident = cb[:, 0:128]          # I128 bf16
            w1a = cb[:, 128:256]          # [128,128] W1cat rows 0:128
            w1b = cb[0:72, 256:384]       # [72,128] W1cat rows 128:200
            w2 = cb[:, 384:448]           # [128,64] [W2r;W2l]
            w3 = cb[:, 448:512]           # [128,64]
            wp = cb[:, 512:513]           # [128,1] [Wpo;Wpr]
            wlin = cb[0:65, 516:518]      # [65,2] [Wlin;blin]
            identf = cf[:, 0:128]         # fp32 I128

            pooled_ps = ppers.tile([64, 64], F32, tag="pooled")

            # node-major scores: s_all[p, 4g+c]
            s_all = cpool.tile([128, 256], F32, tag="sall")
            nc.vector.memset(s_all[:], 0.0)

            # ---------------- per-graph stage emitters ----------------
            state = {}

            def st(g):
                return state.setdefault(g, {})

            def stage_dma(g):
                s = st(g)
                s["xa"] = xpool.tile([128, 400], E4, tag="xa", name="xat")
                nc.sync.dma_start(out=s["xa"][:], in_=xa_d[g])
                s["xb"] = xpool.tile([72, 400], E4, tag="xb", name="xbt")
                nc.sync.dma_start(out=s["xb"][:], in_=xb_d[g])
                s["a"] = apool.tile([128, 1600], E4, tag="a", name="at")
                nc.sync.dma_start(out=s["a"][:], in_=adj_d[g])

            def a_tile(g, kc, mc):
                w = 128 if mc < 3 else 16
                return st(g)["a"][:, kc * 400 + mc * 128: kc * 400 + mc * 128 + w]

            def stage_l1(g):
                # u|v = x @ [W1l|W1r]; u -> bf16 sbuf (DVE), v -> bf16 sbuf (ACT)
                s = st(g)
                puv = puvp.tile([128, 4, 128], F32, tag="puv")
                if puv_init[0] < 2:
                    puv_init[0] += 1
                    nc.vector.memset(puv[:, 3, :], 0.0)
                for c in range(NCH):
                    xs = slice(c * 128, min((c + 1) * 128, 400))
                    out = puv[:, c, :] if c < 3 else puv[0:16, 3, :]
                    nc.tensor.matmul(out, s["xa"][:, xs], w1a,
                                     start=True, stop=False)
                    nc.tensor.matmul(out, s["xb"][:, xs], w1b,
                                     start=False, stop=True)
                u1 = upool.tile([128, 4, H], B16, tag="u1")
                nc.scalar.activation(u1[:], puv[:, :, 0:H], AF.Copy)
                v1 = upool.tile([128, 4, H], B16, tag="v1")
                nc.scalar.activation(v1[:], puv[:, :, H:128], AF.Copy)
                s["u1"], s["v1"] = u1, v1

            def stage_agg(g, src_tile, out_key):
                # sum-aggregation: pA[dst] = sum_src A_raw[src,dst] * src[src]
                s = st(g)
                pA = paggp.tile([128, 4, H], F32, tag="agg")
                if agg_init[0] < 2:
                    agg_init[0] += 1
                    nc.vector.memset(pA[:, 3, :], 0.0)
                for mc in range(NCH):
                    out = pA[:, mc, :] if mc < 3 else pA[0:16, 3, :]
                    for kc in range(NCH):
                        nc.tensor.matmul(out, a_tile(g, kc, mc),
                                         src_tile[:, kc, 0:H],
                                         start=(kc == 0), stop=(kc == NCH - 1))
                s[out_key] = pA

            def invd_ap(g):
                return invd_sb[:, g * 4: g * 4 + 4]

            def ivbc(g):
                return invd_ap(g).unsqueeze(2).broadcast_to([128, 4, H])

            def epi_l1(g):
                # h1 = relu(sum1 * invd + v1): TT (DVE) then add+relu (GpSimd)
                s = st(g)
                tmp = upool.tile([128, 4, H], B16, tag="t1")
                nc.vector.tensor_tensor(tmp[:], s["p1"][:], ivbc(g), OP.mult)
                tmp2 = upool.tile([128, 4, H], B16, tag="t2")
                nc.vector.tensor_tensor(tmp2[:], tmp[:], s["v1"][:], OP.add)
                hcat = hpool.tile([128, 4, 128], B16, tag="hcat")
                nc.vector.tensor_scalar(hcat[:, :, 0:H], tmp2[:], 0.0, None, OP.max)
                s["hc1"] = hcat

            def epi_mean(g, pkey, hckey, last=False):
                # mean-agg copy into cat bottom half (ACT, per-partition scale)
                s = st(g)
                iv = invd_ap(g)
                hc = s[hckey]
                if last:
                    # score layer needs the RAW sum-aggregation (no 1/deg)
                    nc.scalar.activation(hc[:, :, H:128], s[pkey][:], AF.Copy)
                    return
                nc.vector.tensor_tensor(hc[:, :, H:128], s[pkey][:], ivbc(g),
                                        OP.mult)

            def stage_tz(g, hckey, wcat, out_hckey, layer):
                # transpose cat -> zt (GpSimd copy), z matmul, epilogue STT (DVE)
                s = st(g)
                hc = s[hckey]
                pT = pscr.tile([128, 512], B16, tag="ps", name="pT")
                for c in range(NCH):
                    nc.tensor.transpose(pT[:, ts(c, 128)], hc[:, c, :], ident)
                zt = zpool.tile([128, 512], B16, tag="zt")
                if zt_init[0] < 4:
                    zt_init[0] += 1
                    nc.gpsimd.memset(zt[:, 400:512], 0.0)
                nc.scalar.activation(zt[:, 0:400], pT[:, 0:400], AF.Copy)
                pZ = pscr.tile([128, 4, H], F32, tag="ps", name="pZ")
                for c in range(NCH):
                    nc.tensor.matmul(pZ[:, c, :], zt[:, ts(c, 128)], wcat,
                                     start=True, stop=True)
                if layer == 3:
                    hn = kpool.tile([128, 4, 128], B16, tag="h3k")
                    h3list.append(hn)
                else:
                    hn = hpool.tile([128, 4, 128], B16, tag="hcat")
                nc.vector.scalar_tensor_tensor(
                    hn[:, :, 0:H], pZ[:], 0.0, hc[:, :, 0:H], OP.max, OP.add)
                s[out_hckey] = hn

            def stage_score_z(g):
                # scores: s = catS^T @ [Wpo;Wpr] (catS = [h3 | raw sum-agg])
                s = st(g)
                hc = s["hc3"]
                pT = pscr.tile([128, 512], B16, tag="ps", name="pT")
                for c in range(NCH):
                    nc.tensor.transpose(pT[:, ts(c, 128)], hc[:, c, :], ident)
                zt = zpool.tile([128, 512], B16, tag="zt")
                if zt_init[0] < 4:
                    zt_init[0] += 1
                    nc.gpsimd.memset(zt[:, 400:512], 0.0)
                nc.scalar.activation(zt[:, 0:400], pT[:, 0:400], AF.Copy)
                s_ps = pscr.tile([128, 4], F32, tag="ps", name="s_ps")
                for c in range(NCH):
                    nc.tensor.matmul(s_ps[:, c:c + 1], zt[:, ts(c, 128)], wp,
                                     start=True, stop=True)
                nc.vector.tensor_scalar(s_all[:, g:256:64], s_ps[:], 0.0, None,
                                        OP.add)

            # ---------------- interleaved graph-group loop (3-deep) ----------------
            h3list = []
            zt_init = [0]
            agg_init = [0]
            puv_init = [0]
            groups = [list(range(i, i + 3)) for i in range(0, 60, 3)] + [[60, 61, 62, 63]]
            assert sum(len(gr) for gr in groups) == g_count
            for gi, gr in enumerate(groups):
                if gi == 0:
                    for g in gr:
                        stage_dma(g)
                if gi + 1 < len(groups):
                    for g in groups[gi + 1]:
                        stage_dma(g)
                for g in gr:
                    stage_l1(g)
                for g in gr:
                    stage_agg(g, st(g)["u1"], "p1")
                    epi_l1(g)
                for g in gr:
                    stage_agg(g, st(g)["hc1"], "p2")
                    epi_mean(g, "p2", "hc1")
                for g in gr:
                    stage_tz(g, "hc1", w2, "hc2", 2)
                for g in gr:
                    stage_agg(g, st(g)["hc2"], "p3")
                    epi_mean(g, "p3", "hc2")
                for g in gr:
                    stage_tz(g, "hc2", w3, "hc3", 3)
                for g in gr:
                    stage_agg(g, st(g)["hc3"], "pS")
                    epi_mean(g, "pS", "hc3", last=True)
                for g in gr:
                    stage_score_z(g)
                if dbg and 0 in gr:
                    a = 0
                    nc.gpsimd.dma_start(out=dbg_u1[:], in_=st(a)["u1"][:])
                    nc.gpsimd.dma_start(out=dbg_v1[:], in_=st(a)["v1"][:])
                    nc.gpsimd.dma_start(out=dbg_h1[:], in_=st(a)["hc1"][:])
                    nc.gpsimd.dma_start(out=dbg_h2[:], in_=st(a)["hc2"][:])
                    nc.gpsimd.dma_start(out=dbg_h3[:], in_=st(a)["hc3"][:])
                for g in gr:
                    st(g).clear()

            # ------------------------- top-k threshold -------------------------
            S = cpool.tile([64, 512], F32, tag="S")
            for c in range(4):
                pTs = pscr.tile([64, 128], F32, tag="ps", name="pTs")
                nc.tensor.transpose(pTs[:], s_all[:, ts(c, 64)], identf)
                nc.vector.tensor_copy(S[:, ts(c, 128)], pTs[:])
            nc.vector.memset(S[:, 400:512], -1e30)
            ones400 = cpool.tile([64, 400], F32, tag="ones400")
            nc.vector.memset(ones400[:], 1.0)
            cmp_s = cpool.tile([64, 400], F32, tag="cmps")
            lo = cpool.tile([64, 1], F32, tag="lo")
            hi = cpool.tile([64, 1], F32, tag="hi")
            mid = cpool.tile([64, 1], F32, tag="mid")
            cnt = cpool.tile([64, 1], F32, tag="cnt")
            msk = cpool.tile([64, 1], mybir.dt.uint8, tag="msk")
            msk2 = cpool.tile([64, 1], mybir.dt.uint8, tag="msk2")
            nc.vector.tensor_reduce(lo[:], S[:, 0:400], AX, OP.min)
            nc.vector.tensor_scalar(lo[:], lo[:], -1.0, None, OP.add)
            nc.vector.tensor_reduce(hi[:], S[:, 0:400], AX, OP.max)
            nc.vector.tensor_scalar(hi[:], hi[:], 1.0, None, OP.add)
            for _ in range(n_bisect):
                nc.vector.tensor_tensor(mid[:], lo[:], hi[:], OP.add)
                nc.vector.tensor_scalar(mid[:], mid[:], 0.5, None, OP.mult)
                nc.vector.scalar_tensor_tensor(
                    cmp_s[:], S[:, 0:400], mid[:], ones400[:], OP.is_ge, OP.mult,
                    accum_out=cnt[:])
                nc.vector.tensor_scalar(msk[:], cnt[:], 200.0, None, OP.is_ge)
                nc.vector.tensor_scalar(msk2[:], cnt[:], 200.0, None, OP.is_lt)
                nc.vector.select(lo[:], msk[:], mid[:], lo[:])
                nc.vector.select(hi[:], msk2[:], mid[:], hi[:])

            if dbg:
                nc.gpsimd.dma_start(out=dbg_sS[:], in_=S[:])
            # w = tanh(s) * (s >= thresh)   (graph-major)
            tnh = cpool.tile([64, 512], F32, tag="tnh")
            nc.scalar.activation(tnh[:], S[:], AF.Tanh)
            wgm = cpool.tile([64, 512], B16, tag="wgm")
            nc.vector.scalar_tensor_tensor(
                wgm[:], S[:], lo[:], tnh[:], OP.is_ge, OP.mult)
            if dbg:
                nc.gpsimd.dma_start(out=dbg_lo[:], in_=lo[:])
                nc.gpsimd.dma_start(out=dbg_w[:], in_=wgm[:])
            pw = pscr.tile([128, 4, H], B16, tag="ps", name="pw")
            for c in range(NCH):
                nc.tensor.transpose(pw[:, c, :], wgm[:, ts(c, 128)],
                                    ident[0:64, 0:64])
            w_all = cpool.tile([128, 4, H], B16, tag="wall")
            nc.scalar.activation(w_all[:], pw[:], AF.Copy)

            # ------------------------- pooling + classifier --------------------
            if g_count < 64:
                nc.vector.memset(pooled_ps[:], 0.0)
            for g in range(g_count):
                h3k = h3list[g]
                for c in range(NCH):
                    nc.tensor.matmul(pooled_ps[:, g:g + 1], h3k[:, c, 0:H],
                                     w_all[:, c, g:g + 1],
                                     start=(c == 0), stop=(c == NCH - 1))
            pool_fm = cpool.tile([65, 64], B16, tag="poolfm")
            nc.vector.memset(pool_fm[64:65, :], 1.0)
            nc.scalar.activation(pool_fm[0:64, :], pooled_ps[:], AF.Copy,
                                 scale=1.0 / 200.0)
            if dbg:
                nc.gpsimd.dma_start(out=dbg_pf[:], in_=pool_fm[:])
            plw = pscr.tile([1, 128], F32, tag="ps", name="plw")
            for cls in range(2):
                nc.tensor.matmul(plw[0:1, ts(cls, 64)], wlin[:, cls:cls + 1],
                                 pool_fm[:], start=True, stop=True)
            lgw = cpool.tile([1, 128], F32, tag="lgw")
            nc.vector.tensor_copy(lgw[:], plw[:])
            m01 = cpool.tile([1, 64], F32, tag="m01")
            d0 = cpool.tile([1, 64], F32, tag="d0")
            d1 = cpool.tile([1, 64], F32, tag="d1")
            e0 = cpool.tile([1, 64], F32, tag="e0")
            e1 = cpool.tile([1, 64], F32, tag="e1")
            lse = cpool.tile([1, 64], F32, tag="lse")
            out_sb = cpool.tile([1, 128], F32, tag="outsb")
            nc.vector.tensor_tensor(m01[:], lgw[:, 0:64], lgw[:, 64:128], OP.max)
            nc.vector.tensor_tensor(d0[:], lgw[:, 0:64], m01[:], OP.subtract)
            nc.vector.tensor_tensor(d1[:], lgw[:, 64:128], m01[:], OP.subtract)
            nc.scalar.activation(e0[:], d0[:], AF.Exp)
            nc.scalar.activation(e1[:], d1[:], AF.Exp)
            nc.vector.tensor_tensor(lse[:], e0[:], e1[:], OP.add)
            nc.scalar.activation(lse[:], lse[:], AF.Ln)
            nc.vector.tensor_tensor(out_sb[:, 0:64], d0[:], lse[:], OP.subtract)
            nc.vector.tensor_tensor(out_sb[:, 64:128], d1[:], lse[:], OP.subtract)
            ov = out_sb[:].rearrange("p (a b) -> p a b", a=2)[:, :, 0:g_count]
            nc.sync.dma_start(out=out_d[:], in_=ov)

    nc.compile()
    return nc


# ----------------------------------------------------------------------------
# Host-side shard/layout prep
# ----------------------------------------------------------------------------

def _prep(x, edge_index, W1l, W1r, W2l, W2r, W3l, W3r, Wpr, Wpo, Wlin, blin,
          n_graphs=B):
    src = np.asarray(edge_index[0]) % NPG
    dst = np.asarray(edge_index[1]) % NPG
    key = (src.astype(np.int64) * NPG + dst).reshape(n_graphs, EPG)

    A = np.zeros((n_graphs, NPG * NPG), np.float32)
    for g in range(n_graphs):
        A[g] = np.bincount(key[g], minlength=NPG * NPG)
    A = A.reshape(n_graphs, NPG, NPG)          # A[g, s, d] = edge count s->d
    deg = A.sum(axis=1)                        # in-degree per dst [g, 400]
    inv = (1.0 / np.maximum(deg, 1.0)).astype(np.float32)
    Ap = np.zeros((n_graphs, NP, NPG), np.float32)
    Ap[:, :NPG, :] = A
    adj = np.ascontiguousarray(
        Ap.reshape(n_graphs, 4, 128, NPG).transpose(0, 2, 1, 3)
        .reshape(n_graphs, 128, 4 * NPG)).astype(F8)

    invp = np.zeros((n_graphs, NP), np.float32)
    invp[:, :NPG] = inv
    inv_nm = np.ascontiguousarray(
        invp.reshape(n_graphs, 4, 128).transpose(2, 0, 1)
        .reshape(128, n_graphs * 4))

    x = np.asarray(x, np.float32)
    xT = x.reshape(n_graphs, NPG, F_IN).transpose(0, 2, 1)
    xa = np.ascontiguousarray(xT[:, 0:128, :]).astype(F8)
    xb = np.ascontiguousarray(xT[:, 128:200, :]).astype(F8)

    def n_(a):
        return np.asarray(a, np.float32)

    cb16 = np.zeros((128, 520), np.float32)
    cb16[:, 0:128] = np.eye(128)
    w1cat = np.concatenate([n_(W1l), n_(W1r)], axis=1)       # [200, 128]
    cb16[:, 128:256] = w1cat[0:128]
    cb16[0:72, 256:384] = w1cat[128:200]
    cb16[:, 384:448] = np.concatenate([n_(W2r), n_(W2l)], axis=0)
    cb16[:, 448:512] = np.concatenate([n_(W3r), n_(W3l)], axis=0)
    cb16[:, 512:513] = np.concatenate([n_(Wpo), n_(Wpr)], axis=0)
    cb16[0:64, 516:518] = n_(Wlin)
    cb16[64, 516:518] = n_(blin)
    cb16 = cb16.astype(BF16)

    cf32 = np.eye(128, dtype=np.float32)

    return xa, xb, adj, inv_nm, cb16, cf32


def kernel(**inputs):
    xa, xb, adj, inv_nm, cb16, cf32 = _prep(
        inputs["x"], inputs["edge_index"], inputs["W1l"], inputs["W1r"],
        inputs["W2l"], inputs["W2r"], inputs["W3l"], inputs["W3r"],
        inputs["Wpr"], inputs["Wpo"], inputs["Wlin"], inputs["blin"])

    nc = build_kernel(G)

    in_maps = []
    for c in range(NCORES):
        gs = slice(c * G, (c + 1) * G)
        in_maps.append({
            "xa": np.ascontiguousarray(xa[gs]),
            "xb": np.ascontiguousarray(xb[gs]),
            "adj": np.ascontiguousarray(adj[gs]),
            "invd": np.ascontiguousarray(inv_nm[:, c * G * 4:(c + 1) * G * 4]),
            "cb16": cb16,
            "cf32": cf32,
        })
    res = run_bass_kernel_spmd(nc, in_maps, list(range(NCORES)))
    outs = [res.results[i]["out"] for i in range(NCORES)]    # each [2, G]
    logits = np.concatenate(outs, axis=1).T                  # [512, 2]
    return np.ascontiguousarray(logits.astype(np.float32))


# revision 22
# speedup vs baseline: 1.4398x; 1.0363x over previous
"""DeepGraphSAGE Trainium2 kernel (8 NeuronCores, data-parallel over graphs).

v2: fp8 inputs (x, raw-count adjacency), inv-degree in epilogues,
two-graph software pipelining to keep the tensor engine busy, epilogues
spread across Vector/Scalar/GpSimd engines.

Sharding: 512 graphs -> 64 per core; edges never cross graphs. Per graph
the 400x400 adjacency ships as raw edge counts (exact in fp8e4m3) in
PE-tile layout; node features ship transposed fp8; weights replicated.
"""

import sys

sys.path.insert(0, "/opt/trn_rl_repo")

import numpy as np
import ml_dtypes

import concourse.bass as bass
import concourse.bacc as bacc
import concourse.mybir as mybir
from concourse.tile import TileContext
from concourse.bass_utils import run_bass_kernel_spmd

BF16 = ml_dtypes.bfloat16
F8 = ml_dtypes.float8_e4m3fn
F32 = mybir.dt.float32
B16 = mybir.dt.bfloat16
E4 = mybir.dt.float8e4

NCORES = 8
B = 512          # graphs
NPG = 400        # nodes per graph
NP = 512         # padded nodes per graph
EPG = 6400       # edges per graph
F_IN = 200       # input feature dim
H = 64           # hidden
NCH = 4          # node chunks of 128
G = B // NCORES  # graphs per core

AX = mybir.AxisListType.X
OP = mybir.AluOpType
AF = mybir.ActivationFunctionType


def ts(i, n):
    return slice(i * n, (i + 1) * n)


# ----------------------------------------------------------------------------
# Device kernel
# ----------------------------------------------------------------------------

def build_kernel(g_count=G, n_bisect=24, dbg=False):
    nc = bacc.Bacc("TRN2", debug=False)

    xa_d = nc.declare_dram_parameter("xa", [g_count, 128, 400], E4, isOutput=False)
    xb_d = nc.declare_dram_parameter("xb", [g_count, 72, 400], E4, isOutput=False)
    adj_d = nc.declare_dram_parameter("adj", [g_count, 128, 1600], E4, isOutput=False)
    invd_d = nc.declare_dram_parameter("invd", [128, g_count * 4], F32, isOutput=False)
    cb_d = nc.declare_dram_parameter("cb16", [128, 520], B16, isOutput=False)
    cf_d = nc.declare_dram_parameter("cf32", [128, 128], F32, isOutput=False)
    out_d = nc.declare_dram_parameter("out", [2, g_count], F32, isOutput=True)
    if dbg:
        dbg_u1 = nc.declare_dram_parameter("dbg_u1", [128, 256], F32, isOutput=True)
        dbg_v1 = nc.declare_dram_parameter("dbg_v1", [128, 256], F32, isOutput=True)
        dbg_h1 = nc.declare_dram_parameter("dbg_h1", [128, 512], F32, isOutput=True)
        dbg_h2 = nc.declare_dram_parameter("dbg_h2", [128, 512], F32, isOutput=True)
        dbg_h3 = nc.declare_dram_parameter("dbg_h3", [128, 512], F32, isOutput=True)
        dbg_sS = nc.declare_dram_parameter("dbg_sS", [64, 512], F32, isOutput=True)
        dbg_lo = nc.declare_dram_parameter("dbg_lo", [64, 1], F32, isOutput=True)
        dbg_w = nc.declare_dram_parameter("dbg_w", [64, 512], F32, isOutput=True)
        dbg_pf = nc.declare_dram_parameter("dbg_pf", [65, 64], F32, isOutput=True)

    with TileContext(nc) as tc:
        with (
            tc.tile_pool(name="const", bufs=1) as cpool,
            tc.tile_pool(name="xp", bufs=4) as xpool,
            tc.tile_pool(name="ap", bufs=4) as apool,
            tc.tile_pool(name="up", bufs=4) as upool,
            tc.tile_pool(name="hp", bufs=5) as hpool,
            tc.tile_pool(name="zp", bufs=4) as zpool,
            tc.tile_pool(name="kp", bufs=g_count) as kpool,
            tc.tile_pool(name="puvp", bufs=2, space="PSUM") as puvp,
            tc.tile_pool(name="paggp", bufs=2, space="PSUM") as paggp,
            tc.tile_pool(name="pscr", bufs=3, space="PSUM") as pscr,
            tc.tile_pool(name="ppers", bufs=1, space="PSUM") as ppers,
        ):
            # ---- constants ----
            cb = cpool.tile([128, 520], B16, tag="cb")
            nc.sync.dma_start(out=cb[:], in_=cb_d[:])
            cf = cpool.tile([128, 128], F32, tag="cf")
            nc.sync.dma_start(out=cf[:], in_=cf_d[:])
            invd_sb = cpool.tile([128, g_count * 4], F32, tag="invd")
            nc.sync.dma_start(out=invd_sb[:], in_=invd_d[:])

            ident = cb[:, 0:128]          # I128 bf16
            w1a = cb[:, 128:256]          # [128,128] W1cat rows 0:128
            w1b = cb[0:72, 256:384]       # [72,128] W1cat rows 128:200
            w2 = cb[:, 384:448]           # [128,64] [W2r;W2l]
            w3 = cb[:, 448:512]           # [128,64]
            wp = cb[:, 512:513]           # [128,1] [Wpo;Wpr]
            wlin = cb[0:65, 516:518]      # [65,2] [Wlin;blin]
            identf = cf[:, 0:128]         # fp32 I128

            pooled_ps = ppers.tile([64, 64], F32, tag="pooled")

            # node-major scores: s_all[p, 4g+c]
            s_all = cpool.tile([128, 256], F32, tag="sall")
            nc.vector.memset(s_all[:], 0.0)

            # ---------------- per-graph stage emitters ----------------
            state = {}

            def st(g):
                return state.setdefault(g, {})

            def stage_dma(g):
                s = st(g)
                s["xa"] = xpool.tile([128, 400], E4, tag="xa", name="xat")
                nc.sync.dma_start(out=s["xa"][:], in_=xa_d[g])
                s["xb"] = xpool.tile([72, 400], E4, tag="xb", name="xbt")
                nc.sync.dma_start(out=s["xb"][:], in_=xb_d[g])
                s["a"] = apool.tile([128, 1600], E4, tag="a", name="at")
                nc.sync.dma_start(out=s["a"][:], in_=adj_d[g])

            def a_tile(g, kc, mc):
                w = 128 if mc < 3 else 16
                return st(g)["a"][:, kc * 400 + mc * 128: kc * 400 + mc * 128 + w]

            def stage_l1(g):
                # u|v = x @ [W1l|W1r]; u -> bf16 sbuf (DVE), v -> bf16 sbuf (ACT)
                s = st(g)
                puv = puvp.tile([128, 4, 128], F32, tag="puv")
                if puv_init[0] < 2:
                    puv_init[0] += 1
                    nc.vector.memset(puv[:, 3, :], 0.0)
                for c in range(NCH):
                    xs = slice(c * 128, min((c + 1) * 128, 400))
                    out = puv[:, c, :] if c < 3 else puv[0:16, 3, :]
                    nc.tensor.matmul(out, s["xa"][:, xs], w1a,
                                     start=True, stop=False)
                    nc.tensor.matmul(out, s["xb"][:, xs], w1b,
                                     start=False, stop=True)
                u1 = upool.tile([128, 4, H], B16, tag="u1")
                nc.scalar.activation(u1[:], puv[:, :, 0:H], AF.Copy)
                v1 = upool.tile([128, 4, H], B16, tag="v1")
                nc.scalar.activation(v1[:], puv[:, :, H:128], AF.Copy)
                s["u1"], s["v1"] = u1, v1

            def stage_agg(g, src_tile, out_key):
                # sum-aggregation: pA[dst] = sum_src A_raw[src,dst] * src[src]
                s = st(g)
                pA = paggp.tile([128, 4, H], F32, tag="agg")
                if agg_init[0] < 2:
                    agg_init[0] += 1
                    nc.vector.memset(pA[:, 3, :], 0.0)
                for mc in range(NCH):
                    out = pA[:, mc, :] if mc < 3 else pA[0:16, 3, :]
                    for kc in range(NCH):
                        nc.tensor.matmul(out, a_tile(g, kc, mc),
                                         src_tile[:, kc, 0:H],
                                         start=(kc == 0), stop=(kc == NCH - 1))
                s[out_key] = pA

            def invd_ap(g):
                return invd_sb[:, g * 4: g * 4 + 4]

            def ivbc(g):
                return invd_ap(g).unsqueeze(2).broadcast_to([128, 4, H])

            def epi_l1(g):
                # h1 = relu(sum1 * invd + v1): TT (DVE) then add+relu (GpSimd)
                s = st(g)
                tmp = upool.tile([128, 4, H], B16, tag="t1")
                nc.vector.tensor_tensor(tmp[:], s["p1"][:], ivbc(g), OP.mult)
                tmp2 = upool.tile([128, 4, H], B16, tag="t2")
                nc.vector.tensor_tensor(tmp2[:], tmp[:], s["v1"][:], OP.add)
                hcat = hpool.tile([128, 4, 128], B16, tag="hcat")
                nc.vector.tensor_scalar(hcat[:, :, 0:H], tmp2[:], 0.0, None, OP.max)
                s["hc1"] = hcat

            def epi_mean(g, pkey, hckey, last=False):
                # mean-agg copy into cat bottom half (ACT, per-partition scale)
                s = st(g)
                iv = invd_ap(g)
                hc = s[hckey]
                if last:
                    # score layer needs the RAW sum-aggregation (no 1/deg)
                    nc.scalar.activation(hc[:, :, H:128], s[pkey][:], AF.Copy)
                    return
                nc.vector.tensor_tensor(hc[:, :, H:128], s[pkey][:], ivbc(g),
                                        OP.mult)

            def stage_tz(g, hckey, wcat, out_hckey, layer):
                # transpose cat -> zt (GpSimd copy), z matmul, epilogue STT (DVE)
                s = st(g)
                hc = s[hckey]
                pT = pscr.tile([128, 512], B16, tag="ps", name="pT")
                for c in range(NCH):
                    nc.tensor.transpose(pT[:, ts(c, 128)], hc[:, c, :], ident)
                zt = zpool.tile([128, 512], B16, tag="zt")
                if zt_init[0] < 4:
                    zt_init[0] += 1
                    nc.gpsimd.memset(zt[:, 400:512], 0.0)
                nc.scalar.activation(zt[:, 0:400], pT[:, 0:400], AF.Copy)
                pZ = pscr.tile([128, 4, H], F32, tag="ps", name="pZ")
                for c in range(NCH):
                    nc.tensor.matmul(pZ[:, c, :], zt[:, ts(c, 128)], wcat,
                                     start=True, stop=True)
                if layer == 3:
                    hn = kpool.tile([128, 4, 128], B16, tag="h3k")
                    h3list.append(hn)
                else:
                    hn = hpool.tile([128, 4, 128], B16, tag="hcat")
                nc.vector.scalar_tensor_tensor(
                    hn[:, :, 0:H], pZ[:], 0.0, hc[:, :, 0:H], OP.max, OP.add)
                s[out_hckey] = hn

            def stage_score_z(g):
                # scores: s = catS^T @ [Wpo;Wpr] (catS = [h3 | raw sum-agg])
                s = st(g)
                hc = s["hc3"]
                pT = pscr.tile([128, 512], B16, tag="ps", name="pT")
                for c in range(NCH):
                    nc.tensor.transpose(pT[:, ts(c, 128)], hc[:, c, :], ident)
                zt = zpool.tile([128, 512], B16, tag="zt")
                if zt_init[0] < 4:
                    zt_init[0] += 1
                    nc.gpsimd.memset(zt[:, 400:512], 0.0)
                nc.scalar.activation(zt[:, 0:400], pT[:, 0:400], AF.Copy)
                s_ps = pscr.tile([128, 4], F32, tag="ps", name="s_ps")
                for c in range(NCH):
                    nc.tensor.matmul(s_ps[:, c:c + 1], zt[:, ts(c, 128)], wp,
                                     start=True, stop=True)
                nc.vector.tensor_scalar(s_all[:, g:256:64], s_ps[:], 0.0, None,
                                        OP.add)

            # ---- batched top-k threshold (32 graphs per batch) ----
            tk = {}

            def topk_part1(b):
                r0 = 32 * b
                Sb = cpool.tile([32, 512], F32, tag=f"S{b}", name=f"S{b}")
                for c in range(4):
                    pTs = pscr.tile([32, 128], F32, tag="ps", name="pTs")
                    nc.tensor.transpose(pTs[:], s_all[:, c * 64 + r0:
                                                      c * 64 + r0 + 32], identf)
                    nc.vector.tensor_copy(Sb[:, ts(c, 128)], pTs[:])
                nc.vector.memset(Sb[:, 400:512], -1e30)
                cmp_s = cpool.tile([32, 400], F32, tag=f"cmps{b}", name=f"cm{b}")
                ones = cpool.tile([32, 400], F32, tag=f"ones{b}", name=f"on{b}")
                nc.vector.memset(ones[:], 1.0)
                lo = cpool.tile([32, 1], F32, tag=f"lo{b}", name=f"lo{b}")
                cnt = cpool.tile([32, 1], F32, tag=f"cnt{b}", name=f"cnt{b}")
                mid = cpool.tile([32, 1], F32, tag=f"mid{b}", name=f"mid{b}")
                msk = cpool.tile([32, 1], mybir.dt.uint8, tag=f"msk{b}",
                                 name=f"msk{b}")
                nc.vector.tensor_reduce(lo[:], Sb[:, 0:400], AX, OP.min)
                nc.vector.tensor_scalar(lo[:], lo[:], -0.5, None, OP.add)
                for i in range(n_bisect):
                    step = 256.0 / (2 ** (i + 1))
                    nc.vector.tensor_scalar(mid[:], lo[:], step, None, OP.add)
                    nc.vector.scalar_tensor_tensor(
                        cmp_s[:], Sb[:, 0:400], mid[:], ones[:], OP.is_ge,
                        OP.mult, accum_out=cnt[:])
                    nc.vector.tensor_scalar(msk[:], cnt[:], 200.0, None, OP.is_ge)
                    nc.vector.select(lo[:], msk[:], mid[:], lo[:])
                tnh = cpool.tile([32, 512], F32, tag=f"tnh{b}", name=f"tnh{b}")
                nc.scalar.activation(tnh[:], Sb[:], AF.Tanh)
                wgm = cpool.tile([32, 512], B16, tag=f"wgm{b}", name=f"wgm{b}")
                nc.vector.scalar_tensor_tensor(wgm[:], Sb[:], lo[:], tnh[:],
                                               OP.is_ge, OP.mult)
                tk[b] = wgm
                if dbg and b == 0:
                    nc.gpsimd.dma_start(out=dbg_sS[0:32, :], in_=Sb[:])
                    nc.gpsimd.dma_start(out=dbg_lo[0:32, :], in_=lo[:])
                    nc.gpsimd.dma_start(out=dbg_w[0:32, :], in_=wgm[:])

            def pool_part2(b):
                r0 = 32 * b
                wgm = tk[b]
                pw = pscr.tile([128, 4, 32], B16, tag="ps", name="pw")
                for c in range(NCH):
                    nc.tensor.transpose(pw[:, c, :], wgm[:, ts(c, 128)],
                                        ident[0:32, 0:32])
                wab = cpool.tile([128, 4, 32], B16, tag=f"wall{b}",
                                 name=f"wall{b}")
                nc.scalar.activation(wab[:], pw[:], AF.Copy)
                for g in range(r0, r0 + 32):
                    h3k = h3list[g]
                    for c in range(NCH):
                        nc.tensor.matmul(pooled_ps[:, g:g + 1], h3k[:, c, 0:H],
                                         wab[:, c, g - r0:g - r0 + 1],
                                         start=(c == 0), stop=(c == NCH - 1))

            # ---------------- interleaved graph-group loop (3-deep) ----------------
            h3list = []
            zt_init = [0]
            agg_init = [0]
            puv_init = [0]
            groups = [list(range(i, i + 3)) for i in range(0, 60, 3)] + [[60, 61, 62, 63]]
            assert sum(len(gr) for gr in groups) == g_count
            for gi, gr in enumerate(groups):
                if gi == 0:
                    for g in gr:
                        stage_dma(g)
                if gi + 1 < len(groups):
                    for g in groups[gi + 1]:
                        stage_dma(g)
                for g in gr:
                    stage_l1(g)
                for g in gr:
                    stage_agg(g, st(g)["u1"], "p1")
                    epi_l1(g)
                for g in gr:
                    stage_agg(g, st(g)["hc1"], "p2")
                    epi_mean(g, "p2", "hc1")
                for g in gr:
                    stage_tz(g, "hc1", w2, "hc2", 2)
                for g in gr:
                    stage_agg(g, st(g)["hc2"], "p3")
                    epi_mean(g, "p3", "hc2")
                for g in gr:
                    stage_tz(g, "hc2", w3, "hc3", 3)
                for g in gr:
                    stage_agg(g, st(g)["hc3"], "pS")
                    epi_mean(g, "pS", "hc3", last=True)
                for g in gr:
                    stage_score_z(g)
                if dbg and 0 in gr:
                    a = 0
                    nc.gpsimd.dma_start(out=dbg_u1[:], in_=st(a)["u1"][:])
                    nc.gpsimd.dma_start(out=dbg_v1[:], in_=st(a)["v1"][:])
                    nc.gpsimd.dma_start(out=dbg_h1[:], in_=st(a)["hc1"][:])
                    nc.gpsimd.dma_start(out=dbg_h2[:], in_=st(a)["hc2"][:])
                    nc.gpsimd.dma_start(out=dbg_h3[:], in_=st(a)["hc3"][:])
                for g in gr:
                    st(g).clear()
                if gi == 10:
                    topk_part1(0)
                if gi == 17:
                    pool_part2(0)

            # -------- batch 1 topk (tail) + pooling + classifier --------
            topk_part1(1)
            pool_part2(1)

            pool_fm = cpool.tile([65, 64], B16, tag="poolfm")
            nc.vector.memset(pool_fm[64:65, :], 1.0)
            nc.scalar.activation(pool_fm[0:64, :], pooled_ps[:], AF.Copy,
                                 scale=1.0 / 200.0)
            if dbg:
                nc.gpsimd.dma_start(out=dbg_pf[:], in_=pool_fm[:])
            plw = pscr.tile([1, 128], F32, tag="ps", name="plw")
            for cls in range(2):
                nc.tensor.matmul(plw[0:1, ts(cls, 64)], wlin[:, cls:cls + 1],
                                 pool_fm[:], start=True, stop=True)
            lgw = cpool.tile([1, 128], F32, tag="lgw")
            nc.vector.tensor_copy(lgw[:], plw[:])
            m01 = cpool.tile([1, 64], F32, tag="m01")
            d0 = cpool.tile([1, 64], F32, tag="d0")
            d1 = cpool.tile([1, 64], F32, tag="d1")
            e0 = cpool.tile([1, 64], F32, tag="e0")
            e1 = cpool.tile([1, 64], F32, tag="e1")
            lse = cpool.tile([1, 64], F32, tag="lse")
            out_sb = cpool.tile([1, 128], F32, tag="outsb")
            nc.vector.tensor_tensor(m01[:], lgw[:, 0:64], lgw[:, 64:128], OP.max)
            nc.vector.tensor_tensor(d0[:], lgw[:, 0:64], m01[:], OP.subtract)
            nc.vector.tensor_tensor(d1[:], lgw[:, 64:128], m01[:], OP.subtract)
            nc.scalar.activation(e0[:], d0[:], AF.Exp)
            nc.scalar.activation(e1[:], d1[:], AF.Exp)
            nc.vector.tensor_tensor(lse[:], e0[:], e1[:], OP.add)
            nc.scalar.activation(lse[:], lse[:], AF.Ln)
            nc.vector.tensor_tensor(out_sb[:, 0:64], d0[:], lse[:], OP.subtract)
            nc.vector.tensor_tensor(out_sb[:, 64:128], d1[:], lse[:], OP.subtract)
            ov = out_sb[:].rearrange("p (a b) -> p a b", a=2)[:, :, 0:g_count]
            nc.sync.dma_start(out=out_d[:], in_=ov)

    nc.compile()
    return nc


# ----------------------------------------------------------------------------
# Host-side shard/layout prep
# ----------------------------------------------------------------------------

def _prep(x, edge_index, W1l, W1r, W2l, W2r, W3l, W3r, Wpr, Wpo, Wlin, blin,
          n_graphs=B):
    src = np.asarray(edge_index[0]) % NPG
    dst = np.asarray(edge_index[1]) % NPG
    key = (src.astype(np.int64) * NPG + dst).reshape(n_graphs, EPG)

    A = np.zeros((n_graphs, NPG * NPG), np.float32)
    for g in range(n_graphs):
        A[g] = np.bincount(key[g], minlength=NPG * NPG)
    A = A.reshape(n_graphs, NPG, NPG)          # A[g, s, d] = edge count s->d
    deg = A.sum(axis=1)                        # in-degree per dst [g, 400]
    inv = (1.0 / np.maximum(deg, 1.0)).astype(np.float32)
    Ap = np.zeros((n_graphs, NP, NPG), np.float32)
    Ap[:, :NPG, :] = A
    adj = np.ascontiguousarray(
        Ap.reshape(n_graphs, 4, 128, NPG).transpose(0, 2, 1, 3)
        .reshape(n_graphs, 128, 4 * NPG)).astype(F8)

    invp = np.zeros((n_graphs, NP), np.float32)
    invp[:, :NPG] = inv
    inv_nm = np.ascontiguousarray(
        invp.reshape(n_graphs, 4, 128).transpose(2, 0, 1)
        .reshape(128, n_graphs * 4))

    x = np.asarray(x, np.float32)
    xT = x.reshape(n_graphs, NPG, F_IN).transpose(0, 2, 1)
    xa = np.ascontiguousarray(xT[:, 0:128, :]).astype(F8)
    xb = np.ascontiguousarray(xT[:, 128:200, :]).astype(F8)

    def n_(a):
        return np.asarray(a, np.float32)

    cb16 = np.zeros((128, 520), np.float32)
    cb16[:, 0:128] = np.eye(128)
    w1cat = np.concatenate([n_(W1l), n_(W1r)], axis=1)       # [200, 128]
    cb16[:, 128:256] = w1cat[0:128]
    cb16[0:72, 256:384] = w1cat[128:200]
    cb16[:, 384:448] = np.concatenate([n_(W2r), n_(W2l)], axis=0)
    cb16[:, 448:512] = np.concatenate([n_(W3r), n_(W3l)], axis=0)
    cb16[:, 512:513] = np.concatenate([n_(Wpo), n_(Wpr)], axis=0)
    cb16[0:64, 516:518] = n_(Wlin)
    cb16[64, 516:518] = n_(blin)
    cb16 = cb16.astype(BF16)

    cf32 = np.eye(128, dtype=np.float32)

    return xa, xb, adj, inv_nm, cb16, cf32


def kernel(**inputs):
    xa, xb, adj, inv_nm, cb16, cf32 = _prep(
        inputs["x"], inputs["edge_index"], inputs["W1l"], inputs["W1r"],
        inputs["W2l"], inputs["W2r"], inputs["W3l"], inputs["W3r"],
        inputs["Wpr"], inputs["Wpo"], inputs["Wlin"], inputs["blin"])

    nc = build_kernel(G)

    in_maps = []
    for c in range(NCORES):
        gs = slice(c * G, (c + 1) * G)
        in_maps.append({
            "xa": np.ascontiguousarray(xa[gs]),
            "xb": np.ascontiguousarray(xb[gs]),
            "adj": np.ascontiguousarray(adj[gs]),
            "invd": np.ascontiguousarray(inv_nm[:, c * G * 4:(c + 1) * G * 4]),
            "cb16": cb16,
            "cf32": cf32,
        })
    res = run_bass_kernel_spmd(nc, in_maps, list(range(NCORES)))
    outs = [res.results[i]["out"] for i in range(NCORES)]    # each [2, G]
    logits = np.concatenate(outs, axis=1).T                  # [512, 2]
    return np.ascontiguousarray(logits.astype(np.float32))


# revision 23
# speedup vs baseline: 1.5315x; 1.0637x over previous
"""DeepGraphSAGE Trainium2 kernel (8 NeuronCores, data-parallel over graphs).

v2: fp8 inputs (x, raw-count adjacency), inv-degree in epilogues,
two-graph software pipelining to keep the tensor engine busy, epilogues
spread across Vector/Scalar/GpSimd engines.

Sharding: 512 graphs -> 64 per core; edges never cross graphs. Per graph
the 400x400 adjacency ships as raw edge counts (exact in fp8e4m3) in
PE-tile layout; node features ship transposed fp8; weights replicated.
"""

import sys

sys.path.insert(0, "/opt/trn_rl_repo")

import numpy as np
import ml_dtypes

import concourse.bass as bass
import concourse.bacc as bacc
import concourse.mybir as mybir
from concourse.tile import TileContext
from concourse.bass_utils import run_bass_kernel_spmd

BF16 = ml_dtypes.bfloat16
F8 = ml_dtypes.float8_e4m3fn
F32 = mybir.dt.float32
B16 = mybir.dt.bfloat16
E4 = mybir.dt.float8e4

NCORES = 8
B = 512          # graphs
NPG = 400        # nodes per graph
NP = 512         # padded nodes per graph
EPG = 6400       # edges per graph
F_IN = 200       # input feature dim
H = 64           # hidden
NCH = 4          # node chunks of 128
G = B // NCORES  # graphs per core

AX = mybir.AxisListType.X
OP = mybir.AluOpType
AF = mybir.ActivationFunctionType


def ts(i, n):
    return slice(i * n, (i + 1) * n)


# ----------------------------------------------------------------------------
# Device kernel
# ----------------------------------------------------------------------------

def build_kernel(g_count=G, n_bisect=24, dbg=False):
    nc = bacc.Bacc("TRN2", debug=False)

    xa_d = nc.declare_dram_parameter("xa", [g_count, 128, 400], E4, isOutput=False)
    xb_d = nc.declare_dram_parameter("xb", [g_count, 72, 400], E4, isOutput=False)
    adj_d = nc.declare_dram_parameter("adj", [g_count, 128, 1600], E4, isOutput=False)
    invd_d = nc.declare_dram_parameter("invd", [128, g_count * 4], F32, isOutput=False)
    cb_d = nc.declare_dram_parameter("cb16", [128, 520], B16, isOutput=False)
    cf_d = nc.declare_dram_parameter("cf32", [128, 128], F32, isOutput=False)
    out_d = nc.declare_dram_parameter("out", [2, g_count], F32, isOutput=True)
    if dbg:
        dbg_u1 = nc.declare_dram_parameter("dbg_u1", [128, 256], F32, isOutput=True)
        dbg_v1 = nc.declare_dram_parameter("dbg_v1", [128, 256], F32, isOutput=True)
        dbg_h1 = nc.declare_dram_parameter("dbg_h1", [128, 512], F32, isOutput=True)
        dbg_h2 = nc.declare_dram_parameter("dbg_h2", [128, 512], F32, isOutput=True)
        dbg_h3 = nc.declare_dram_parameter("dbg_h3", [128, 512], F32, isOutput=True)
        dbg_sS = nc.declare_dram_parameter("dbg_sS", [64, 512], F32, isOutput=True)
        dbg_lo = nc.declare_dram_parameter("dbg_lo", [64, 1], F32, isOutput=True)
        dbg_w = nc.declare_dram_parameter("dbg_w", [64, 512], F32, isOutput=True)
        dbg_pf = nc.declare_dram_parameter("dbg_pf", [65, 64], F32, isOutput=True)

    with TileContext(nc) as tc:
        with (
            tc.tile_pool(name="const", bufs=1) as cpool,
            tc.tile_pool(name="xp", bufs=4) as xpool,
            tc.tile_pool(name="ap", bufs=4) as apool,
            tc.tile_pool(name="up", bufs=4) as upool,
            tc.tile_pool(name="hp", bufs=6) as hpool,
            tc.tile_pool(name="zp", bufs=4) as zpool,
            tc.tile_pool(name="kp", bufs=g_count) as kpool,
            tc.tile_pool(name="puvp", bufs=2, space="PSUM") as puvp,
            tc.tile_pool(name="paggp", bufs=2, space="PSUM") as paggp,
            tc.tile_pool(name="pscr", bufs=3, space="PSUM") as pscr,
            tc.tile_pool(name="ppers", bufs=1, space="PSUM") as ppers,
        ):
            # ---- constants ----
            cb = cpool.tile([128, 520], B16, tag="cb")
            nc.sync.dma_start(out=cb[:], in_=cb_d[:])
            cf = cpool.tile([128, 128], F32, tag="cf")
            nc.sync.dma_start(out=cf[:], in_=cf_d[:])
            invd_sb = cpool.tile([128, g_count * 4], F32, tag="invd")
            nc.sync.dma_start(out=invd_sb[:], in_=invd_d[:])

            ident = cb[:, 0:128]          # I128 bf16
            w1a = cb[:, 128:256]          # [128,128] W1cat rows 0:128
            w1b = cb[0:72, 256:384]       # [72,128] W1cat rows 128:200
            w2 = cb[:, 384:448]           # [128,64] [W2r;W2l]
            w3 = cb[:, 448:512]           # [128,64]
            wp = cb[:, 512:513]           # [128,1] [Wpo;Wpr]
            wlin = cb[0:65, 516:518]      # [65,2] [Wlin;blin]
            identf = cf[:, 0:128]         # fp32 I128

            pooled_ps = ppers.tile([64, 64], F32, tag="pooled")

            # node-major scores: s_all[p, 4g+c]
            s_all = cpool.tile([128, 256], F32, tag="sall")
            nc.vector.memset(s_all[:], 0.0)

            # ---------------- per-graph stage emitters ----------------
            state = {}

            def st(g):
                return state.setdefault(g, {})

            def stage_dma(g):
                s = st(g)
                s["xa"] = xpool.tile([128, 400], E4, tag="xa", name="xat")
                nc.sync.dma_start(out=s["xa"][:], in_=xa_d[g])
                s["xb"] = xpool.tile([72, 400], E4, tag="xb", name="xbt")
                nc.sync.dma_start(out=s["xb"][:], in_=xb_d[g])
                s["a"] = apool.tile([128, 1600], E4, tag="a", name="at")
                nc.sync.dma_start(out=s["a"][:], in_=adj_d[g])

            def a_tile(g, kc, mc):
                w = 128 if mc < 3 else 16
                return st(g)["a"][:, kc * 400 + mc * 128: kc * 400 + mc * 128 + w]

            def stage_l1(g):
                # u|v = x @ [W1l|W1r]; u -> bf16 sbuf (DVE), v -> bf16 sbuf (ACT)
                s = st(g)
                puv = puvp.tile([128, 4, 128], F32, tag="puv")
                if puv_init[0] < 2:
                    puv_init[0] += 1
                    nc.vector.memset(puv[:, 3, :], 0.0)
                for c in range(NCH):
                    xs = slice(c * 128, min((c + 1) * 128, 400))
                    out = puv[:, c, :] if c < 3 else puv[0:16, 3, :]
                    nc.tensor.matmul(out, s["xa"][:, xs], w1a,
                                     start=True, stop=False)
                    nc.tensor.matmul(out, s["xb"][:, xs], w1b,
                                     start=False, stop=True)
                u1 = upool.tile([128, 4, H], B16, tag="u1")
                nc.scalar.activation(u1[:], puv[:, :, 0:H], AF.Copy)
                v1 = upool.tile([128, 4, H], B16, tag="v1")
                nc.scalar.activation(v1[:], puv[:, :, H:128], AF.Copy)
                s["u1"], s["v1"] = u1, v1

            def stage_agg(g, src_tile, out_key):
                # sum-aggregation: pA[dst] = sum_src A_raw[src,dst] * src[src]
                s = st(g)
                pA = paggp.tile([128, 4, H], F32, tag="agg")
                if agg_init[0] < 2:
                    agg_init[0] += 1
                    nc.vector.memset(pA[:, 3, :], 0.0)
                for mc in range(NCH):
                    out = pA[:, mc, :] if mc < 3 else pA[0:16, 3, :]
                    for kc in range(NCH):
                        nc.tensor.matmul(out, a_tile(g, kc, mc),
                                         src_tile[:, kc, 0:H],
                                         start=(kc == 0), stop=(kc == NCH - 1))
                s[out_key] = pA

            def invd_ap(g):
                return invd_sb[:, g * 4: g * 4 + 4]

            def ivbc(g):
                return invd_ap(g).unsqueeze(2).broadcast_to([128, 4, H])

            def epi_l1(g):
                # h1 = relu(sum1 * invd + v1): TT (DVE) then add+relu (GpSimd)
                s = st(g)
                tmp = upool.tile([128, 4, H], B16, tag="t1")
                nc.vector.tensor_tensor(tmp[:], s["p1"][:], ivbc(g), OP.mult)
                tmp2 = upool.tile([128, 4, H], B16, tag="t2")
                nc.vector.tensor_tensor(tmp2[:], tmp[:], s["v1"][:], OP.add)
                hcat = hpool.tile([128, 4, 128], B16, tag="hcat")
                nc.vector.tensor_scalar(hcat[:, :, 0:H], tmp2[:], 0.0, None, OP.max)
                s["hc1"] = hcat

            def epi_mean(g, pkey, hckey, last=False):
                # mean-agg copy into cat bottom half (ACT, per-partition scale)
                s = st(g)
                iv = invd_ap(g)
                hc = s[hckey]
                if last:
                    # score layer needs the RAW sum-aggregation (no 1/deg)
                    nc.scalar.activation(hc[:, :, H:128], s[pkey][:], AF.Copy)
                    return
                nc.vector.tensor_tensor(hc[:, :, H:128], s[pkey][:], ivbc(g),
                                        OP.mult)

            def stage_tz(g, hckey, wcat, out_hckey, layer):
                # transpose cat -> zt (GpSimd copy), z matmul, epilogue STT (DVE)
                s = st(g)
                hc = s[hckey]
                pT = pscr.tile([128, 512], B16, tag="ps", name="pT")
                for c in range(NCH):
                    nc.tensor.transpose(pT[:, ts(c, 128)], hc[:, c, :], ident)
                zt = zpool.tile([128, 512], B16, tag="zt")
                if zt_init[0] < 4:
                    zt_init[0] += 1
                    nc.gpsimd.memset(zt[:, 400:512], 0.0)
                nc.scalar.activation(zt[:, 0:400], pT[:, 0:400], AF.Copy)
                pZ = pscr.tile([128, 4, H], F32, tag="ps", name="pZ")
                for c in range(NCH):
                    nc.tensor.matmul(pZ[:, c, :], zt[:, ts(c, 128)], wcat,
                                     start=True, stop=True)
                if layer == 3:
                    hn = kpool.tile([128, 4, 128], B16, tag="h3k")
                    h3list.append(hn)
                else:
                    hn = hpool.tile([128, 4, 128], B16, tag="hcat")
                nc.vector.scalar_tensor_tensor(
                    hn[:, :, 0:H], pZ[:], 0.0, hc[:, :, 0:H], OP.max, OP.add)
                s[out_hckey] = hn

            def stage_score_z(g):
                # scores: s = catS^T @ [Wpo;Wpr] (catS = [h3 | raw sum-agg])
                s = st(g)
                hc = s["hc3"]
                pT = pscr.tile([128, 512], B16, tag="ps", name="pT")
                for c in range(NCH):
                    nc.tensor.transpose(pT[:, ts(c, 128)], hc[:, c, :], ident)
                zt = zpool.tile([128, 512], B16, tag="zt")
                if zt_init[0] < 4:
                    zt_init[0] += 1
                    nc.gpsimd.memset(zt[:, 400:512], 0.0)
                nc.scalar.activation(zt[:, 0:400], pT[:, 0:400], AF.Copy)
                s_ps = pscr.tile([128, 4], F32, tag="ps", name="s_ps")
                for c in range(NCH):
                    nc.tensor.matmul(s_ps[:, c:c + 1], zt[:, ts(c, 128)], wp,
                                     start=True, stop=True)
                nc.vector.tensor_scalar(s_all[:, g:256:64], s_ps[:], 0.0, None,
                                        OP.add)

            # ---- batched top-k threshold (32 graphs per batch) ----
            tk = {}

            def topk_part1(b):
                r0 = 32 * b
                Sb = cpool.tile([32, 512], F32, tag=f"S{b}", name=f"S{b}")
                for c in range(4):
                    pTs = pscr.tile([32, 128], F32, tag="ps", name="pTs")
                    nc.tensor.transpose(pTs[:], s_all[:, c * 64 + r0:
                                                      c * 64 + r0 + 32], identf)
                    nc.vector.tensor_copy(Sb[:, ts(c, 128)], pTs[:])
                nc.vector.memset(Sb[:, 400:512], -1e30)
                cmp_s = cpool.tile([32, 400], F32, tag=f"cmps{b}", name=f"cm{b}")
                ones = cpool.tile([32, 400], F32, tag=f"ones{b}", name=f"on{b}")
                nc.vector.memset(ones[:], 1.0)
                lo = cpool.tile([32, 1], F32, tag=f"lo{b}", name=f"lo{b}")
                cnt = cpool.tile([32, 1], F32, tag=f"cnt{b}", name=f"cnt{b}")
                mid = cpool.tile([32, 1], F32, tag=f"mid{b}", name=f"mid{b}")
                msk = cpool.tile([32, 1], mybir.dt.uint8, tag=f"msk{b}",
                                 name=f"msk{b}")
                nc.vector.tensor_reduce(lo[:], Sb[:, 0:400], AX, OP.min)
                nc.vector.tensor_scalar(lo[:], lo[:], -0.5, None, OP.add)
                for i in range(n_bisect):
                    step = 256.0 / (2 ** (i + 1))
                    nc.vector.tensor_scalar(mid[:], lo[:], step, None, OP.add)
                    nc.vector.scalar_tensor_tensor(
                        cmp_s[:], Sb[:, 0:400], mid[:], ones[:], OP.is_ge,
                        OP.mult, accum_out=cnt[:])
                    nc.vector.tensor_scalar(msk[:], cnt[:], 200.0, None, OP.is_ge)
                    nc.vector.select(lo[:], msk[:], mid[:], lo[:])
                tnh = cpool.tile([32, 512], F32, tag=f"tnh{b}", name=f"tnh{b}")
                nc.scalar.activation(tnh[:], Sb[:], AF.Tanh)
                wgm = cpool.tile([32, 512], B16, tag=f"wgm{b}", name=f"wgm{b}")
                nc.vector.scalar_tensor_tensor(wgm[:], Sb[:], lo[:], tnh[:],
                                               OP.is_ge, OP.mult)
                tk[b] = wgm
                if dbg and b == 0:
                    nc.gpsimd.dma_start(out=dbg_sS[0:32, :], in_=Sb[:])
                    nc.gpsimd.dma_start(out=dbg_lo[0:32, :], in_=lo[:])
                    nc.gpsimd.dma_start(out=dbg_w[0:32, :], in_=wgm[:])

            def pool_part2(b):
                r0 = 32 * b
                wgm = tk[b]
                pw = pscr.tile([128, 4, 32], B16, tag="ps", name="pw")
                for c in range(NCH):
                    nc.tensor.transpose(pw[:, c, :], wgm[:, ts(c, 128)],
                                        ident[0:32, 0:32])
                wab = cpool.tile([128, 4, 32], B16, tag=f"wall{b}",
                                 name=f"wall{b}")
                nc.scalar.activation(wab[:], pw[:], AF.Copy)
                for g in range(r0, r0 + 32):
                    h3k = h3list[g]
                    for c in range(NCH):
                        nc.tensor.matmul(pooled_ps[:, g:g + 1], h3k[:, c, 0:H],
                                         wab[:, c, g - r0:g - r0 + 1],
                                         start=(c == 0), stop=(c == NCH - 1))

            # ---------------- interleaved graph-group loop (3-deep) ----------------
            h3list = []
            zt_init = [0]
            agg_init = [0]
            puv_init = [0]
            groups = [list(range(i, i + 4)) for i in range(0, 64, 4)]
            assert sum(len(gr) for gr in groups) == g_count
            for gi, gr in enumerate(groups):
                if gi == 0:
                    for g in gr:
                        stage_dma(g)
                if gi + 1 < len(groups):
                    for g in groups[gi + 1]:
                        stage_dma(g)
                for g in gr:
                    stage_l1(g)
                for g in gr:
                    stage_agg(g, st(g)["u1"], "p1")
                    epi_l1(g)
                for g in gr:
                    stage_agg(g, st(g)["hc1"], "p2")
                    epi_mean(g, "p2", "hc1")
                for g in gr:
                    stage_tz(g, "hc1", w2, "hc2", 2)
                for g in gr:
                    stage_agg(g, st(g)["hc2"], "p3")
                    epi_mean(g, "p3", "hc2")
                for g in gr:
                    stage_tz(g, "hc2", w3, "hc3", 3)
                for g in gr:
                    stage_agg(g, st(g)["hc3"], "pS")
                    epi_mean(g, "pS", "hc3", last=True)
                for g in gr:
                    stage_score_z(g)
                if dbg and 0 in gr:
                    a = 0
                    nc.gpsimd.dma_start(out=dbg_u1[:], in_=st(a)["u1"][:])
                    nc.gpsimd.dma_start(out=dbg_v1[:], in_=st(a)["v1"][:])
                    nc.gpsimd.dma_start(out=dbg_h1[:], in_=st(a)["hc1"][:])
                    nc.gpsimd.dma_start(out=dbg_h2[:], in_=st(a)["hc2"][:])
                    nc.gpsimd.dma_start(out=dbg_h3[:], in_=st(a)["hc3"][:])
                for g in gr:
                    st(g).clear()
                if gi == 8:
                    topk_part1(0)
                if gi == 13:
                    pool_part2(0)

            # -------- batch 1 topk (tail) + pooling + classifier --------
            topk_part1(1)
            pool_part2(1)

            pool_fm = cpool.tile([65, 64], B16, tag="poolfm")
            nc.vector.memset(pool_fm[64:65, :], 1.0)
            nc.scalar.activation(pool_fm[0:64, :], pooled_ps[:], AF.Copy,
                                 scale=1.0 / 200.0)
            if dbg:
                nc.gpsimd.dma_start(out=dbg_pf[:], in_=pool_fm[:])
            plw = pscr.tile([1, 128], F32, tag="ps", name="plw")
            for cls in range(2):
                nc.tensor.matmul(plw[0:1, ts(cls, 64)], wlin[:, cls:cls + 1],
                                 pool_fm[:], start=True, stop=True)
            lgw = cpool.tile([1, 128], F32, tag="lgw")
            nc.vector.tensor_copy(lgw[:], plw[:])
            m01 = cpool.tile([1, 64], F32, tag="m01")
            d0 = cpool.tile([1, 64], F32, tag="d0")
            d1 = cpool.tile([1, 64], F32, tag="d1")
            e0 = cpool.tile([1, 64], F32, tag="e0")
            e1 = cpool.tile([1, 64], F32, tag="e1")
            lse = cpool.tile([1, 64], F32, tag="lse")
            out_sb = cpool.tile([1, 128], F32, tag="outsb")
            nc.vector.tensor_tensor(m01[:], lgw[:, 0:64], lgw[:, 64:128], OP.max)
            nc.vector.tensor_tensor(d0[:], lgw[:, 0:64], m01[:], OP.subtract)
            nc.vector.tensor_tensor(d1[:], lgw[:, 64:128], m01[:], OP.subtract)
            nc.scalar.activation(e0[:], d0[:], AF.Exp)
            nc.scalar.activation(e1[:], d1[:], AF.Exp)
            nc.vector.tensor_tensor(lse[:], e0[:], e1[:], OP.add)
            nc.scalar.activation(lse[:], lse[:], AF.Ln)
            nc.vector.tensor_tensor(out_sb[:, 0:64], d0[:], lse[:], OP.subtract)
            nc.vector.tensor_tensor(out_sb[:, 64:128], d1[:], lse[:], OP.subtract)
            ov = out_sb[:].rearrange("p (a b) -> p a b", a=2)[:, :, 0:g_count]
            nc.sync.dma_start(out=out_d[:], in_=ov)

    nc.compile()
    return nc


# ----------------------------------------------------------------------------
# Host-side shard/layout prep
# ----------------------------------------------------------------------------

def _prep(x, edge_index, W1l, W1r, W2l, W2r, W3l, W3r, Wpr, Wpo, Wlin, blin,
          n_graphs=B):
    src = np.asarray(edge_index[0]) % NPG
    dst = np.asarray(edge_index[1]) % NPG
    key = (src.astype(np.int64) * NPG + dst).reshape(n_graphs, EPG)

    A = np.zeros((n_graphs, NPG * NPG), np.float32)
    for g in range(n_graphs):
        A[g] = np.bincount(key[g], minlength=NPG * NPG)
    A = A.reshape(n_graphs, NPG, NPG)          # A[g, s, d] = edge count s->d
    deg = A.sum(axis=1)                        # in-degree per dst [g, 400]
    inv = (1.0 / np.maximum(deg, 1.0)).astype(np.float32)
    Ap = np.zeros((n_graphs, NP, NPG), np.float32)
    Ap[:, :NPG, :] = A
    adj = np.ascontiguousarray(
        Ap.reshape(n_graphs, 4, 128, NPG).transpose(0, 2, 1, 3)
        .reshape(n_graphs, 128, 4 * NPG)).astype(F8)

    invp = np.zeros((n_graphs, NP), np.float32)
    invp[:, :NPG] = inv
    inv_nm = np.ascontiguousarray(
        invp.reshape(n_graphs, 4, 128).transpose(2, 0, 1)
        .reshape(128, n_graphs * 4))

    x = np.asarray(x, np.float32)
    xT = x.reshape(n_graphs, NPG, F_IN).transpose(0, 2, 1)
    xa = np.ascontiguousarray(xT[:, 0:128, :]).astype(F8)
    xb = np.ascontiguousarray(xT[:, 128:200, :]).astype(F8)

    def n_(a):
        return np.asarray(a, np.float32)

    cb16 = np.zeros((128, 520), np.float32)
    cb16[:, 0:128] = np.eye(128)
    w1cat = np.concatenate([n_(W1l), n_(W1r)], axis=1)       # [200, 128]
    cb16[:, 128:256] = w1cat[0:128]
    cb16[0:72, 256:384] = w1cat[128:200]
    cb16[:, 384:448] = np.concatenate([n_(W2r), n_(W2l)], axis=0)
    cb16[:, 448:512] = np.concatenate([n_(W3r), n_(W3l)], axis=0)
    cb16[:, 512:513] = np.concatenate([n_(Wpo), n_(Wpr)], axis=0)
    cb16[0:64, 516:518] = n_(Wlin)
    cb16[64, 516:518] = n_(blin)
    cb16 = cb16.astype(BF16)

    cf32 = np.eye(128, dtype=np.float32)

    return xa, xb, adj, inv_nm, cb16, cf32


def kernel(**inputs):
    xa, xb, adj, inv_nm, cb16, cf32 = _prep(
        inputs["x"], inputs["edge_index"], inputs["W1l"], inputs["W1r"],
        inputs["W2l"], inputs["W2r"], inputs["W3l"], inputs["W3r"],
        inputs["Wpr"], inputs["Wpo"], inputs["Wlin"], inputs["blin"])

    nc = build_kernel(G)

    in_maps = []
    for c in range(NCORES):
        gs = slice(c * G, (c + 1) * G)
        in_maps.append({
            "xa": np.ascontiguousarray(xa[gs]),
            "xb": np.ascontiguousarray(xb[gs]),
            "adj": np.ascontiguousarray(adj[gs]),
            "invd": np.ascontiguousarray(inv_nm[:, c * G * 4:(c + 1) * G * 4]),
            "cb16": cb16,
            "cf32": cf32,
        })
    res = run_bass_kernel_spmd(nc, in_maps, list(range(NCORES)))
    outs = [res.results[i]["out"] for i in range(NCORES)]    # each [2, G]
    logits = np.concatenate(outs, axis=1).T                  # [512, 2]
    return np.ascontiguousarray(logits.astype(np.float32))
